# revision 4
# baseline (speedup 1.0000x reference)
"""Grouped attention pooling kernel for Trainium2 (8 NeuronCores, SPMD).

Reference computation (T=2048 agents, 128 sorted groups, d=64):
    Wh = h @ W.T + b
    sigma[i,j] = f[i,j,:] . Wh[j,:]
    scores     = sigma masked to the query's group (self -> -1000, outside -> -inf)
    attn       = softmax(scores, axis=1);  S = attn @ h;  size-1 groups -> 0

segment_ids is sorted, so attention is block-diagonal over groups (mean size
~16): only f[i, lo_g:hi_g, :] is ever needed (~9 MB of the 1 GiB tensor).
The host packs those blocks into per-group "slots" padded to K_pad keys and
K_pad queries; groups are sharded across the 8 cores (data parallel, no
cross-device attention). Every core runs one identical program; only the
packed data differs.

Per-core device program:
  1. WhpackT[d,(slot,k)] = [Wt|b]^T @ [hkeyT|1]    (one PE matmul)
  2. PE-transpose 32-row blocks -> whp_dram[(slot,k), d] row-major
  3. per 128-query tile: broadcast-DMA whp rows across each slot's query
     partitions, multiply with fpack, segmented-reduce over d -> sigma
  4. additive mask, softmax (DVE reduce + ACT exp + per-partition scale)
  5. per-slot attn^T (DVE 32x32 block transpose) @ hkey -> S  (PE, 32x32
     tile_position blocks), DMA out
"""
import os
import numpy as np
from contextlib import ExitStack

import concourse.bass as bass
import concourse.bacc as bacc
import concourse.tile as tile
import concourse.mybir as mybir
from concourse.bass_utils import run_bass_kernel_spmd
from bass_rust import AxisListType

N_CORES = 8
D = 64
NEG = -1.0e30
SELF_MASK = -1000.0
F32 = mybir.dt.float32

LAST_RESULT = None  # BassKernelResults of the most recent run (for test harness)
_PROGRAM_CACHE = {}


def _build_program(K_pad: int, rows: int):
    """One SPMD program, identical across cores. rows = padded rows/core."""
    FREE = K_pad * D
    spt = 128 // K_pad          # slots per 128-row tile
    n_tiles = rows // 128

    nc = bacc.Bacc("TRN2", target_bir_lowering=False, debug=False,
                   enable_asserts=True, num_devices=N_CORES)

    fpack = nc.dram_tensor("fpack", [rows, FREE], F32, kind="ExternalInput")
    hkey = nc.dram_tensor("hkey", [rows, D], F32, kind="ExternalInput")
    hkt_aug = nc.dram_tensor("hkt_aug", [D + 1, rows], F32, kind="ExternalInput")
    wt_aug = nc.dram_tensor("wt_aug", [D + 1, D], F32, kind="ExternalInput")
    m0 = nc.dram_tensor("m0", [rows, K_pad], F32, kind="ExternalInput")
    ident_in = nc.dram_tensor("ident", [64, 64], F32, kind="ExternalInput")
    out = nc.dram_tensor("out", [rows, D], F32, kind="ExternalOutput")

    with tile.TileContext(nc) as tc, ExitStack() as ctx:
        const = ctx.enter_context(tc.tile_pool(name="const", bufs=1))
        small = ctx.enter_context(tc.tile_pool(name="small", bufs=3))
        big = ctx.enter_context(tc.tile_pool(name="big", bufs=2))
        ps = ctx.enter_context(tc.tile_pool(name="ps", bufs=2, space="PSUM"))
        dram = ctx.enter_context(tc.tile_pool(name="dram", bufs=1, space="DRAM"))

        # ---------- Wh packed per (slot, k), row-major in DRAM ----------
        wt_t = const.tile([D + 1, D], F32)
        nc.sync.dma_start(wt_t[:], wt_aug[:])
        hkt_t = const.tile([D + 1, rows], F32)
        nc.sync.dma_start(hkt_t[:], hkt_aug[:])
        ident = const.tile([64, 64], F32)
        nc.sync.dma_start(ident[:], ident_in[:])

        whp_dram = dram.tile([rows, D], F32)
        CH = 512
        for c in range(0, rows, CH):
            cw = min(CH, rows - c)
            whT_ps = ps.tile([D, CH], F32, tag="whT_ps")
            nc.tensor.matmul(whT_ps[:, :cw], wt_t[:], hkt_t[:, c:c + cw],
                             start=True, stop=True)
            whT_sb = small.tile([D, CH], F32, tag="whT_sb")
            nc.scalar.activation(whT_sb[:, :cw], whT_ps[:, :cw],
                                 mybir.ActivationFunctionType.Identity)
            # transpose [64, 128] chunks into row-major [rows, D]
            for rb0 in range(c, c + cw, 128):
                whp_ps = ps.tile([128, D], F32, tag="whp_ps")
                nc.tensor.transpose(whp_ps[:], whT_sb[:, rb0 - c:rb0 - c + 128],
                                    ident[:])
                whp_sb = small.tile([128, D], F32, tag="whp_sb")
                nc.scalar.activation(whp_sb[:], whp_ps[:],
                                     mybir.ActivationFunctionType.Identity)
                nc.sync.dma_start(whp_dram[rb0:rb0 + 128, :], whp_sb[:])

        # ---------- per 128-query tile ----------
        for t in range(n_tiles):
            r0 = t * 128
            ft = big.tile([128, FREE], F32, tag="ft")
            nc.sync.dma_start(ft[:], fpack[r0:r0 + 128, :])

            whb = big.tile([128, FREE], F32, tag="whb")
            for j in range(spt):
                s0 = (t * spt + j) * K_pad
                src = whp_dram[s0:s0 + K_pad, :].flatten().unsqueeze(0) \
                    .broadcast_to((K_pad, FREE))
                nc.sync.dma_start(whb[j * K_pad:(j + 1) * K_pad, :], src)

            m0_t = small.tile([128, K_pad], F32, tag="m0_t")
            nc.sync.dma_start(m0_t[:], m0[r0:r0 + 128, :])
            hk_t = small.tile([128, D], F32, tag="hk_t")
            nc.sync.dma_start(hk_t[:], hkey[r0:r0 + 128, :])

            prod = big.tile([128, FREE], F32, tag="prod")
            nc.vector.tensor_mul(prod[:], ft[:], whb[:])
            sig = small.tile([128, K_pad], F32, tag="sig")
            nc.vector.tensor_reduce(
                sig[:].unsqueeze(2),
                prod[:].rearrange("p (k d) -> p k d", d=D),
                axis=AxisListType.X, op=mybir.AluOpType.add)

            scores = small.tile([128, K_pad], F32, tag="scores")
            nc.vector.tensor_add(scores[:], sig[:], m0_t[:])

            negmax = small.tile([128, 1], F32, tag="negmax")
            nc.vector.tensor_reduce(negmax[:], scores[:], axis=AxisListType.X,
                                    op=mybir.AluOpType.max, negate=True)
            exps = small.tile([128, K_pad], F32, tag="exps")
            sumexp = small.tile([128, 1], F32, tag="sumexp")
            nc.scalar.activation(exps[:], scores[:],
                                 mybir.ActivationFunctionType.Exp,
                                 bias=negmax[:], scale=1.0, accum_out=sumexp[:])
            rinv = small.tile([128, 1], F32, tag="rinv")
            nc.vector.reciprocal(rinv[:], sumexp[:])
            attn = small.tile([128, K_pad], F32, tag="attn")
            nc.vector.tensor_scalar_mul(attn[:], exps[:], rinv[:])

            s_ps = ps.tile([128, D], F32, tag="s_ps")
            if K_pad == 32:
                attnT = small.tile([128, K_pad], F32, tag="attnT")
                nc.vector.transpose(attnT[:], attn[:])
                for j in range(4):
                    sl = slice(32 * j, 32 * j + 32)
                    nc.tensor.matmul(s_ps[sl, :], attnT[sl, :], hk_t[sl, :],
                                     start=True, stop=True,
                                     tile_position=(32 * j, 32 * j))
            else:  # K_pad == 64: PE transpose per slot
                for j in range(spt):
                    sl = slice(64 * j, 64 * j + 64)
                    aT_ps = ps.tile([64, 64], F32, tag="aT_ps")
                    nc.tensor.transpose(aT_ps[:], attn[sl, :], ident[:],
                                        tile_position=(64 * j, 0))
                    aT_sb = small.tile([64, 64], F32, tag="aT_sb")
                    nc.scalar.activation(aT_sb[:], aT_ps[:],
                                         mybir.ActivationFunctionType.Identity)
                    nc.tensor.matmul(s_ps[sl, :], aT_sb[:], hk_t[sl, :],
                                     start=True, stop=True,
                                     tile_position=(0, 64 * j))

            s_sb = small.tile([128, D], F32, tag="s_sb")
            nc.scalar.activation(s_sb[:], s_ps[:],
                                 mybir.ActivationFunctionType.Identity)
            nc.sync.dma_start(out[r0:r0 + 128, :], s_sb[:])

    nc.compile()
    return nc


def _plan(seg):
    T = seg.shape[0]
    change = np.nonzero(np.diff(seg))[0] + 1
    starts = np.concatenate([[0], change]).astype(np.int64)
    ends = np.concatenate([change, [T]]).astype(np.int64)
    sizes = ends - starts
    smax = int(sizes.max())
    if smax <= 32:
        K_pad = 32
    elif smax <= 64:
        K_pad = 64
    else:
        raise NotImplementedError(f"group size {smax} > 64")
    G = len(starts)
    S_dev = -(-G // N_CORES)
    rows = -(-(S_dev * K_pad) // 128) * 128
    return starts, ends, sizes, G, K_pad, S_dev, rows


def _pack(f, h, seg, W, b):
    starts, ends, sizes, G, K_pad, S_dev, rows = _plan(seg)
    FREE = K_pad * D
    wt_aug = np.concatenate([W.T, b[None, :]], axis=0)  # [65, 64]
    ident = np.eye(64, dtype=np.float32)

    in_maps = []
    for dev in range(N_CORES):
        g0 = dev * S_dev
        fpack = np.zeros((rows, FREE), dtype=np.float32)
        hkey = np.zeros((rows, D), dtype=np.float32)
        hkt_aug = np.zeros((D + 1, rows), dtype=np.float32)
        hkt_aug[D, :] = 1.0
        m0 = np.full((rows, K_pad), NEG, dtype=np.float32)
        for j in range(S_dev):
            g = g0 + j
            if g >= G:
                break
            lo, hi, s = starts[g], ends[g], int(sizes[g])
            r = j * K_pad
            fpack[r:r + s, :s * D] = f[lo:hi, lo:hi, :].reshape(s, s * D)
            hkey[r:r + s, :] = h[lo:hi, :]
            hkt_aug[:D, r:r + s] = h[lo:hi, :].T
            m0[r:r + s, :s] = 0.0
            m0[np.arange(r, r + s), np.arange(s)] = SELF_MASK
        in_maps.append({"fpack": fpack, "hkey": hkey, "hkt_aug": hkt_aug,
                        "wt_aug": wt_aug, "m0": m0, "ident": ident})
    meta = (starts, ends, sizes, G, K_pad, S_dev, rows)
    return in_maps, meta


def _unpack(per_core_out, meta, T):
    starts, ends, sizes, G, K_pad, S_dev, rows = meta
    outf = np.zeros((T, D), dtype=np.float32)
    for dev in range(N_CORES):
        o = per_core_out[dev]
        g0 = dev * S_dev
        for j in range(S_dev):
            g = g0 + j
            if g >= G:
                break
            if sizes[g] > 1:
                outf[starts[g]:ends[g], :] = o[j * K_pad:j * K_pad + int(sizes[g]), :]
    return outf


def kernel(f, h, segment_ids, W, b):
    global LAST_RESULT
    f = np.asarray(f, dtype=np.float32)
    h = np.asarray(h, dtype=np.float32)
    seg = np.asarray(segment_ids)
    W = np.asarray(W, dtype=np.float32)
    b = np.asarray(b, dtype=np.float32)
    T = h.shape[0]

    in_maps, meta = _pack(f, h, seg, W, b)
    K_pad, rows = meta[4], meta[6]

    key = (K_pad, rows)
    if key not in _PROGRAM_CACHE:
        _PROGRAM_CACHE[key] = _build_program(K_pad, rows)
    nc = _PROGRAM_CACHE[key]

    res = run_bass_kernel_spmd(nc, in_maps, core_ids=list(range(N_CORES)))
    LAST_RESULT = res
    return _unpack([res.results[dev]["out"] for dev in range(N_CORES)], meta, T)


# revision 5
# speedup vs baseline: 1.5465x; 1.5465x over previous
"""Grouped attention pooling kernel for Trainium2 (8 NeuronCores, SPMD).

Reference computation (T=2048 agents, 128 sorted groups, d=64):
    Wh = h @ W.T + b
    sigma[i,j] = f[i,j,:] . Wh[j,:]
    scores     = sigma masked to the query's group (self -> -1000, outside -> -inf)
    attn       = softmax(scores, axis=1);  S = attn @ h;  size-1 groups -> 0

segment_ids is sorted, so attention is block-diagonal over groups (mean size
~16): only f[i, lo_g:hi_g, :] is ever needed (~9 MB of the 1 GiB tensor).
The host packs those blocks into per-group "slots" padded to K_pad keys and
K_pad queries; groups are sharded across the 8 cores (data parallel, no
cross-device attention). Every core runs one identical program; only the
packed data differs.

Device layout trick: f blocks are packed TRANSPOSED (keys on partitions,
(query, d) along free), so the Wh operand of the sigma multiply is a plain
[128, 64] tile broadcast along the free dim with a stride-0 access pattern —
no replicated Wh DMA traffic at all. The [k, q] sigma is flipped back to
[q, k] with a DVE 32x32 block transpose, which also matches the per-slot
tile_position matmuls that compute attn @ h.

Per-core device program:
  1. WhpackT[d,(slot,k)] = [Wt|b]^T @ [hkeyT|1]    (one PE matmul per 512 cols)
  2. PE-transpose 128-col chunks -> whp_sb[(slot,k), d] in SBUF (persistent)
  3. per 128-row tile: fpackT * broadcast(whp_sb) (GpSimd), segmented
     d-reduce (DVE) -> sigmaT[k, q]; DVE block-transpose -> sigma[q, k]
  4. additive mask, softmax (DVE reduce + ACT exp + per-partition scale)
  5. per-slot attn^T (DVE block transpose) @ hkey -> S (PE 32x32
     tile_position blocks), DMA out
"""
import numpy as np
from contextlib import ExitStack

import concourse.bass as bass
import concourse.bacc as bacc
import concourse.tile as tile
import concourse.mybir as mybir
from concourse.bass_utils import run_bass_kernel_spmd
from bass_rust import AxisListType

N_CORES = 8
D = 64
NEG = -1.0e30
SELF_MASK = -1000.0
F32 = mybir.dt.float32

LAST_RESULT = None  # BassKernelResults of the most recent run (for test harness)
_PROGRAM_CACHE = {}

# engine that runs the big [128, K_pad*64] multiply, per tile index (tunable)
MUL_ENGINE = ["gpsimd", "vector", "gpsimd", "vector"]


def _build_program(K_pad: int, rows: int):
    """One SPMD program, identical across cores. rows = padded rows/core."""
    FREE = K_pad * D
    spt = 128 // K_pad          # slots per 128-row tile
    n_tiles = rows // 128

    nc = bacc.Bacc("TRN2", target_bir_lowering=False, debug=False,
                   enable_asserts=True, num_devices=N_CORES)

    fpackt = nc.dram_tensor("fpackt", [rows, FREE], F32, kind="ExternalInput")
    hkey = nc.dram_tensor("hkey", [rows, D], F32, kind="ExternalInput")
    hkt_aug = nc.dram_tensor("hkt_aug", [D + 1, rows], F32, kind="ExternalInput")
    wt_aug = nc.dram_tensor("wt_aug", [D + 1, D], F32, kind="ExternalInput")
    m0 = nc.dram_tensor("m0", [rows, K_pad], F32, kind="ExternalInput")
    ident_in = nc.dram_tensor("ident", [64, 64], F32, kind="ExternalInput")
    out = nc.dram_tensor("out", [rows, D], F32, kind="ExternalOutput")

    with tile.TileContext(nc) as tc, ExitStack() as ctx:
        const = ctx.enter_context(tc.tile_pool(name="const", bufs=1))
        small = ctx.enter_context(tc.tile_pool(name="small", bufs=3))
        big = ctx.enter_context(tc.tile_pool(name="big", bufs=3))
        ps = ctx.enter_context(tc.tile_pool(name="ps", bufs=2, space="PSUM"))

        # ---------- Wh packed per (slot, k), kept in SBUF ----------
        wt_t = const.tile([D + 1, D], F32)
        nc.sync.dma_start(wt_t[:], wt_aug[:])
        hkt_t = const.tile([D + 1, rows], F32)
        nc.sync.dma_start(hkt_t[:], hkt_aug[:])
        ident = const.tile([64, 64], F32)
        nc.sync.dma_start(ident[:], ident_in[:])

        whp_sb = const.tile([128, n_tiles * D], F32)  # [(slot,k) % 128, tile*d]
        CH = 512
        for c in range(0, rows, CH):
            cw = min(CH, rows - c)
            whT_ps = ps.tile([D, CH], F32, tag="whT_ps")
            nc.tensor.matmul(whT_ps[:, :cw], wt_t[:], hkt_t[:, c:c + cw],
                             start=True, stop=True)
            whT_sb = small.tile([D, CH], F32, tag="whT_sb")
            nc.scalar.activation(whT_sb[:, :cw], whT_ps[:, :cw],
                                 mybir.ActivationFunctionType.Identity)
            for rb0 in range(c, c + cw, 128):
                whp_ps = ps.tile([128, D], F32, tag="whp_ps")
                nc.tensor.transpose(whp_ps[:], whT_sb[:, rb0 - c:rb0 - c + 128],
                                    ident[:])
                t = rb0 // 128
                nc.scalar.activation(whp_sb[:, t * D:(t + 1) * D], whp_ps[:],
                                     mybir.ActivationFunctionType.Identity)

        # ---------- per 128-row tile ----------
        for t in range(n_tiles):
            r0 = t * 128
            dma_eng = nc.sync if t % 2 == 0 else nc.scalar
            ft = big.tile([128, FREE], F32, tag="ft")
            dma_eng.dma_start(ft[:], fpackt[r0:r0 + 128, :])

            m0_t = small.tile([128, K_pad], F32, tag="m0_t")
            nc.scalar.dma_start(m0_t[:], m0[r0:r0 + 128, :])
            hk_t = small.tile([128, D], F32, tag="hk_t")
            nc.scalar.dma_start(hk_t[:], hkey[r0:r0 + 128, :])

            # sigmaT[k, q] = sum_d fT[k, (q,d)] * Wh[k, d]
            prod = big.tile([128, FREE], F32, tag="prod")
            whb = whp_sb[:, t * D:(t + 1) * D].unsqueeze(1) \
                .broadcast_to((128, K_pad, D))
            mul_eng = getattr(nc, MUL_ENGINE[t % len(MUL_ENGINE)])
            mul_eng.tensor_mul(prod[:].rearrange("p (q d) -> p q d", d=D),
                               ft[:].rearrange("p (q d) -> p q d", d=D), whb)
            sigT = small.tile([128, K_pad], F32, tag="sigT")
            nc.vector.tensor_reduce(
                sigT[:].unsqueeze(2),
                prod[:].rearrange("p (q d) -> p q d", d=D),
                axis=AxisListType.X, op=mybir.AluOpType.add)

            sig = small.tile([128, K_pad], F32, tag="sig")
            nc.vector.transpose(sig[:], sigT[:])

            scores = small.tile([128, K_pad], F32, tag="scores")
            nc.gpsimd.tensor_add(scores[:], sig[:], m0_t[:])

            negmax = small.tile([128, 1], F32, tag="negmax")
            nc.vector.tensor_reduce(negmax[:], scores[:], axis=AxisListType.X,
                                    op=mybir.AluOpType.max, negate=True)
            exps = small.tile([128, K_pad], F32, tag="exps")
            sumexp = small.tile([128, 1], F32, tag="sumexp")
            nc.scalar.activation(exps[:], scores[:],
                                 mybir.ActivationFunctionType.Exp,
                                 bias=negmax[:], scale=1.0, accum_out=sumexp[:])
            rinv = small.tile([128, 1], F32, tag="rinv")
            nc.vector.reciprocal(rinv[:], sumexp[:])
            attn = small.tile([128, K_pad], F32, tag="attn")
            nc.vector.tensor_scalar_mul(attn[:], exps[:], rinv[:])

            s_ps = ps.tile([128, D], F32, tag="s_ps")
            if K_pad == 32:
                attnT = small.tile([128, K_pad], F32, tag="attnT")
                nc.vector.transpose(attnT[:], attn[:])
                for j in range(4):
                    sl = slice(32 * j, 32 * j + 32)
                    nc.tensor.matmul(s_ps[sl, :], attnT[sl, :], hk_t[sl, :],
                                     start=True, stop=True,
                                     tile_position=(32 * j, 32 * j))
            else:  # K_pad == 64: PE transpose per slot
                for j in range(spt):
                    sl = slice(64 * j, 64 * j + 64)
                    aT_ps = ps.tile([64, 64], F32, tag="aT_ps")
                    nc.tensor.transpose(aT_ps[:], attn[sl, :], ident[:],
                                        tile_position=(64 * j, 0))
                    aT_sb = small.tile([64, 64], F32, tag="aT_sb")
                    nc.scalar.activation(aT_sb[:], aT_ps[:],
                                         mybir.ActivationFunctionType.Identity)
                    nc.tensor.matmul(s_ps[sl, :], aT_sb[:], hk_t[sl, :],
                                     start=True, stop=True,
                                     tile_position=(0, 64 * j))

            s_sb = small.tile([128, D], F32, tag="s_sb")
            nc.scalar.activation(s_sb[:], s_ps[:],
                                 mybir.ActivationFunctionType.Identity)
            nc.sync.dma_start(out[r0:r0 + 128, :], s_sb[:])

    nc.compile()
    return nc


def _plan(seg):
    T = seg.shape[0]
    change = np.nonzero(np.diff(seg))[0] + 1
    starts = np.concatenate([[0], change]).astype(np.int64)
    ends = np.concatenate([change, [T]]).astype(np.int64)
    sizes = ends - starts
    smax = int(sizes.max())
    if smax <= 32:
        K_pad = 32
    elif smax <= 64:
        K_pad = 64
    else:
        raise NotImplementedError(f"group size {smax} > 64")
    G = len(starts)
    S_dev = -(-G // N_CORES)
    rows = -(-(S_dev * K_pad) // 128) * 128
    return starts, ends, sizes, G, K_pad, S_dev, rows


def _pack(f, h, seg, W, b):
    starts, ends, sizes, G, K_pad, S_dev, rows = _plan(seg)
    FREE = K_pad * D
    wt_aug = np.concatenate([W.T, b[None, :]], axis=0)  # [65, 64]
    ident = np.eye(64, dtype=np.float32)

    in_maps = []
    for dev in range(N_CORES):
        g0 = dev * S_dev
        fpackt = np.zeros((rows, FREE), dtype=np.float32)
        hkey = np.zeros((rows, D), dtype=np.float32)
        hkt_aug = np.zeros((D + 1, rows), dtype=np.float32)
        hkt_aug[D, :] = 1.0
        m0 = np.full((rows, K_pad), NEG, dtype=np.float32)
        for j in range(S_dev):
            g = g0 + j
            if g >= G:
                break
            lo, hi, s = starts[g], ends[g], int(sizes[g])
            r = j * K_pad
            blk = f[lo:hi, lo:hi, :]                      # [q, k, d]
            fpackt[r:r + s, :s * D] = blk.transpose(1, 0, 2).reshape(s, s * D)
            hkey[r:r + s, :] = h[lo:hi, :]
            hkt_aug[:D, r:r + s] = h[lo:hi, :].T
            m0[r:r + s, :s] = 0.0
            m0[np.arange(r, r + s), np.arange(s)] = SELF_MASK
        in_maps.append({"fpackt": fpackt, "hkey": hkey, "hkt_aug": hkt_aug,
                        "wt_aug": wt_aug, "m0": m0, "ident": ident})
    meta = (starts, ends, sizes, G, K_pad, S_dev, rows)
    return in_maps, meta


def _unpack(per_core_out, meta, T):
    starts, ends, sizes, G, K_pad, S_dev, rows = meta
    outf = np.zeros((T, D), dtype=np.float32)
    for dev in range(N_CORES):
        o = per_core_out[dev]
        g0 = dev * S_dev
        for j in range(S_dev):
            g = g0 + j
            if g >= G:
                break
            if sizes[g] > 1:
                outf[starts[g]:ends[g], :] = o[j * K_pad:j * K_pad + int(sizes[g]), :]
    return outf


def kernel(f, h, segment_ids, W, b):
    global LAST_RESULT
    f = np.asarray(f, dtype=np.float32)
    h = np.asarray(h, dtype=np.float32)
    seg = np.asarray(segment_ids)
    W = np.asarray(W, dtype=np.float32)
    b = np.asarray(b, dtype=np.float32)
    T = h.shape[0]

    in_maps, meta = _pack(f, h, seg, W, b)
    K_pad, rows = meta[4], meta[6]

    key = (K_pad, rows)
    if key not in _PROGRAM_CACHE:
        _PROGRAM_CACHE[key] = _build_program(K_pad, rows)
    nc = _PROGRAM_CACHE[key]

    res = run_bass_kernel_spmd(nc, in_maps, core_ids=list(range(N_CORES)))
    LAST_RESULT = res
    return _unpack([res.results[dev]["out"] for dev in range(N_CORES)], meta, T)
